# revision 34
# baseline (speedup 1.0000x reference)
"""Multi-head attention (B=4, S=2048, D=1024, 16 heads x 64) on 8 trn2 cores.

Sharding: core = 2*b + g  (b: batch 0..3 data-parallel, g: head-group 0..1
tensor-parallel over 8 heads each).  Each core computes a partial
out[b] = softmax(q k^T / 8) v @ Wo[heads_g] (+ biases); the host sums the
two partials per batch.

Pipeline structure (491 us baseline -> 413 us):
  - x arrives in 8 di-chunk tiles; k/q projection for pair 0 runs
    di-outer (k all 4 sc + q sc0/1) paced by the chunk DMAs
  - v projection emitted just-in-time inside attention(0) sc0's t-loop;
    q sc2/3 + kq of pair p+1 emitted as filler items inside the
    attention t-loops, so the PE has independent work while it would
    otherwise stall on exp (ScalarE) results, and HAM stays at 8/8
  - ScalarE is the critical path: 256 exp activations over [128,1024]
    at ~1.34 us each ~= 343 us; the PE (~360 us of slots including
    drain overlap) hides underneath it
  - PSUM: scps 2x[128,1024] (4 banks) + afps 2x[65,512] (2) +
    mps 2x[128,512] (2) = 8 banks
"""

import sys
import functools

sys.path.insert(0, "/opt/trn_rl_repo")

import numpy as np

B, S, D = 4, 2048, 1024
NHEAD, HD = 16, 64
HLOC = 8          # heads per core
NPAIR = 4         # head pairs per core
NCORES = 8
NT = S // 128     # 16 t-blocks

TRACE = False     # test harness can flip this for profiling
LAST = {}         # exec_time_ns etc. from the most recent run


def _build():
    import concourse.tile as tile
    from concourse import bacc, mybir

    f32 = mybir.dt.float32
    f16 = mybir.dt.float16
    EXP = mybir.ActivationFunctionType.Exp

    nc = bacc.Bacc(None)

    xT_d = nc.dram_tensor("xT", [D, S], f16, kind="ExternalInput")
    wq_d = nc.dram_tensor("wq", [D, HLOC * HD], f16, kind="ExternalInput")
    wk_d = nc.dram_tensor("wk", [D, HLOC * HD], f16, kind="ExternalInput")
    wv_d = nc.dram_tensor("wv", [D, HLOC * HD], f16, kind="ExternalInput")
    wo_d = nc.dram_tensor("wo", [HLOC * HD, D], f16, kind="ExternalInput")
    bq_d = nc.dram_tensor("bq", [128, NPAIR], f32, kind="ExternalInput")
    bk_d = nc.dram_tensor("bk", [128, NPAIR], f32, kind="ExternalInput")
    bv_d = nc.dram_tensor("bv", [1, HLOC * HD], f16, kind="ExternalInput")
    ones_d = nc.dram_tensor("ones", [1, 512], f16, kind="ExternalInput")
    out_d = nc.dram_tensor("out", [S, D], f32, kind="ExternalOutput")

    wkr = wk_d.rearrange("(a p) (b c) -> a b p c", p=128, c=128)
    wqr = wq_d.rearrange("(a p) (b c) -> a b p c", p=128, c=128)
    wvr = wv_d.rearrange("(a p) c -> a p c", p=128)
    xTr = xT_d.rearrange("(n p) s -> n p s", p=128)
    wor = wo_d.rearrange("(p q) d -> p q d", q=128)

    with tile.TileContext(nc) as tc:
        with (
            tc.tile_pool(name="const", bufs=1) as const,
            tc.tile_pool(name="qkvres", bufs=1) as qkvres,
            tc.tile_pool(name="wp", bufs=20) as wp,
            tc.tile_pool(name="afp", bufs=1) as afp,
            tc.tile_pool(name="wop", bufs=1) as wop,
            tc.tile_pool(name="expp", bufs=4) as expp,
            tc.tile_pool(name="small", bufs=4) as small,
            tc.tile_pool(name="ostage", bufs=2) as ostage,
            tc.tile_pool(name="mps", bufs=2, space="PSUM") as mps,
            tc.tile_pool(name="scps", bufs=2, space="PSUM") as scps,
            tc.tile_pool(name="afps", bufs=2, space="PSUM") as afps,
        ):
            # ---- staged DMAs: pair-0 weights + x chunks first -------------
            wkq0 = []
            for di in range(8):
                wk_i = wp.tile([128, 128], f16, tag="wk0", name="wk0")
                nc.sync.dma_start(wk_i[:], wkr[di, 0])
                wq_i = wp.tile([128, 128], f16, tag="wq0", name="wq0")
                nc.sync.dma_start(wq_i[:], wqr[di, 0])
                wkq0.append((wk_i, wq_i))

            xt = []
            for di in range(8):
                xt_i = qkvres.tile([128, S], f16, name=f"xt{di}")
                nc.sync.dma_start(xt_i[:], xTr[di])
                xt.append(xt_i)

            ones = const.tile([1, 512], f16)
            nc.sync.dma_start(ones[:], ones_d[:])
            bqs = const.tile([128, NPAIR], f32)
            nc.sync.dma_start(bqs[:], bq_d[:])
            bks = const.tile([128, NPAIR], f32)
            nc.sync.dma_start(bks[:], bk_d[:])
            bvs = const.tile([1, HLOC * HD], f16)
            nc.sync.dma_start(bvs[:], bv_d[:])
            vcol = const.tile([128, HLOC], f32)
            nc.vector.memset(vcol[:], 1.0)

            wv_t = []
            for di in range(8):
                w_i = wp.tile([128, 512], f16, tag="w", name="wv_i")
                nc.sync.dma_start(w_i[:], wvr[di])
                wv_t.append(w_i)

            qT = [qkvres.tile([128, S], f16, name=f"qT{p}") for p in range(NPAIR)]
            kT = [qkvres.tile([128, S], f16, name=f"kT{p}") for p in range(NPAIR)]
            v_sb = [
                qkvres.tile([128, HLOC, HD + 1], f16, name=f"v{t}")
                for t in range(NT)
            ]
            af_sb = [afp.tile([128, S], f16, name=f"af{p}") for p in range(NPAIR)]
            wo_sb = wop.tile([128, NPAIR, D], f16)

            # ---- kq(0): di-outer, paced by the x chunk DMAs ---------------
            # k (all 4 sc, two scps tiles) + q sc0/1 (two mps tiles) in one
            # x sweep; q sc2/3 deferred to fillers inside attention(0).
            kg = [
                scps.tile([128, 1024], f32, tag="sc", name=f"kg{g}")
                for g in range(2)
            ]
            qg0 = [
                mps.tile([128, 512], f32, tag="ps", name=f"qg{sc}")
                for sc in range(2)
            ]
            for di in range(8):
                for g in range(2):
                    for h in range(2):
                        sc = 2 * g + h
                        nc.tensor.matmul(
                            kg[g][:, h * 512 : (h + 1) * 512],
                            wkq0[di][0][:],
                            xt[di][:, sc * 512 : (sc + 1) * 512],
                            start=(di == 0),
                            stop=(di == 7),
                        )
                for sc in range(2):
                    nc.tensor.matmul(
                        qg0[sc][:],
                        wkq0[di][1][:],
                        xt[di][:, sc * 512 : (sc + 1) * 512],
                        start=(di == 0),
                        stop=(di == 7),
                    )
            for g in range(2):
                for h in range(2):
                    sc = 2 * g + h
                    nc.vector.tensor_add(
                        kT[0][:, sc * 512 : (sc + 1) * 512],
                        kg[g][:, h * 512 : (h + 1) * 512],
                        bks[:, 0:1].broadcast_to([128, 512]),
                    )
            for sc in range(2):
                nc.vector.tensor_add(
                    qT[0][:, sc * 512 : (sc + 1) * 512],
                    qg0[sc][:],
                    bqs[:, 0:1].broadcast_to([128, 512]),
                )

            # ---- filler items ---------------------------------------------
            def v_block(t):
                """v projection for t-block t -> v_sb[t] (+ ones column)."""
                vp = mps.tile([128, 512], f32, tag="ps", name="vp")
                for di in range(8):
                    nc.tensor.matmul(
                        vp[:],
                        xt[di][:, t * 128 : (t + 1) * 128],
                        wv_t[di][:],
                        start=(di == 0),
                        stop=False,
                    )
                nc.tensor.matmul(
                    vp[:], ones[0:1, 0:128], bvs[:], start=False, stop=True
                )
                nc.vector.tensor_copy(
                    v_sb[t][:, :, 0:64],
                    vp[:].rearrange("p (n h) -> p n h", h=64),
                )
                nc.vector.tensor_copy(v_sb[t][:, :, 64:65], vcol[:, :, None])

            def proj_half(st, pp, wsel, sc, half, wblk):
                """Half of a 512-col k/q projection chunk for pair pp; the
                PSUM accumulator is shared between the two halves via st so
                each filler pop stays under ~1us of PE time and never
                delays the next scores by more than that."""
                dest, bias = ((kT, bks), (qT, bqs))[wsel]
                if half == 0:
                    st["p"] = mps.tile([128, 512], f32, tag="ps", name="pj")
                p = st["p"]
                for di in range(4 * half, 4 * half + 4):
                    nc.tensor.matmul(
                        p[:],
                        wblk[di][wsel][:],
                        xt[di][:, sc * 512 : (sc + 1) * 512],
                        start=(di == 0),
                        stop=(di == 7),
                    )
                if half == 1:
                    nc.vector.tensor_add(
                        dest[pp][:, sc * 512 : (sc + 1) * 512],
                        p[:],
                        bias[:, pp : pp + 1].broadcast_to([128, 512]),
                    )

            def proj_halves(pp, wsel, sc, wblk):
                st = {}
                return [
                    functools.partial(proj_half, st, pp, wsel, sc, h, wblk)
                    for h in range(2)
                ]

            def q0_items(sc):
                return proj_halves(0, 1, sc, wkq0)

            def make_kq_filler(pp):
                """DMA + projection half-items for pair pp (k then q)."""
                wblk = []
                for di in range(8):
                    wk_i = wp.tile([128, 128], f16, tag="wkn", name="wkN")
                    nc.sync.dma_start(wk_i[:], wkr[di, pp])
                    wq_i = wp.tile([128, 128], f16, tag="wqn", name="wqN")
                    nc.sync.dma_start(wq_i[:], wqr[di, pp])
                    wblk.append((wk_i, wq_i))
                items = []
                for wsel in range(2):
                    for sc in range(4):
                        items += proj_halves(pp, wsel, sc, wblk)
                return items

            def wo_half(st, sc, si, dch):
                """Half (one 512-wide output chunk) of a Wo output block."""
                s0 = sc * 512 + si * 128
                if dch == 0:
                    st["ot"] = ostage.tile([128, D], f32, tag="ost", name="ot")
                ot = st["ot"]
                op = mps.tile([128, 512], f32, tag="ps", name="op")
                for pp in range(NPAIR):
                    nc.tensor.matmul(
                        op[:],
                        af_sb[pp][:, s0 : s0 + 128],
                        wo_sb[:, pp, dch * 512 : (dch + 1) * 512],
                        start=(pp == 0),
                        stop=(pp == NPAIR - 1),
                    )
                nc.vector.tensor_copy(
                    ot[:, dch * 512 : (dch + 1) * 512], op[:]
                )
                if dch == 1:
                    nc.sync.dma_start(out_d[s0 : s0 + 128, :], ot[:])

            def wo_items(sc, si):
                st = {}
                return [
                    functools.partial(wo_half, st, sc, si, d) for d in range(2)
                ]

            # ---- attention with interleaved fillers -----------------------
            def attention_pair(pp, filler, unlock):
                fi = [0]

                def pop_filler():
                    if fi[0] < len(filler):
                        filler[fi[0]]()
                        fi[0] += 1

                slots = 3 * NT
                # pair 3 starts with an empty list that norm(sc) extends
                # with Wo half-items; poll those at a fixed cadence
                stride = max(1, slots // len(filler)) if filler else 2
                for sc in range(4):
                    ss = sc * 512
                    af0 = afps.tile([65, 512], f32, tag="afps", name="af0")
                    af1 = afps.tile([65, 512], f32, tag="afps", name="af1")
                    for t in range(NT):
                        scp = scps.tile([128, 1024], f32, tag="sc", name="scp")
                        nc.tensor.matmul(
                            scp[:, 0:512],
                            kT[pp][0:64, t * 128 : (t + 1) * 128],
                            qT[pp][0:64, ss : ss + 512],
                            start=True,
                            stop=True,
                            tile_position=(0, 0),
                        )
                        nc.tensor.matmul(
                            scp[:, 512:1024],
                            kT[pp][64:128, t * 128 : (t + 1) * 128],
                            qT[pp][64:128, ss : ss + 512],
                            start=True,
                            stop=True,
                            tile_position=(64, 0),
                        )
                        ex = expp.tile([128, 1024], f16, tag="ex", name="ex")
                        nc.scalar.activation(ex[:], scp[:], EXP, scale=0.125)
                        if pp == 0 and sc == 0:
                            v_block(t)  # just-in-time v for attention(0)
                        nc.tensor.matmul(
                            af0[:],
                            v_sb[t][:, 2 * pp, :],
                            ex[:, 0:512],
                            start=(t == 0),
                            stop=(t == NT - 1),
                        )
                        nc.tensor.matmul(
                            af1[:],
                            v_sb[t][:, 2 * pp + 1, :],
                            ex[:, 512:1024],
                            start=(t == 0),
                            stop=(t == NT - 1),
                        )
                        # interleave filler items across sc 1..3
                        if sc > 0:
                            if ((sc - 1) * NT + t) % stride == stride - 1:
                                pop_filler()
                    for rr, afx in ((0, af0), (1, af1)):
                        zrow = small.tile([1, 512], f32, tag="zrow", name="zrow")
                        nc.vector.tensor_copy(zrow[:], afx[64:65, :])
                        zb = small.tile([64, 512], f32, tag="zb", name="zb")
                        nc.gpsimd.partition_broadcast(zb[:], zrow[:])
                        rec64 = small.tile([64, 512], f32, tag="rec", name="rec64")
                        nc.vector.reciprocal_approx_fast(rec64[:], zb[:])
                        nc.vector.tensor_mul(
                            af_sb[pp][64 * rr : 64 * rr + 64, ss : ss + 512],
                            afx[0:64, :],
                            rec64[:],
                        )
                    filler.extend(unlock(sc))
                while fi[0] < len(filler):
                    filler[fi[0]]()
                    fi[0] += 1

            for pp in range(NPAIR):
                nc.sync.dma_start(wo_sb[:, pp, :], wor[pp])

            def no_unlock(sc):
                return []

            for pp in range(NPAIR):
                filler = []
                if pp == 0:
                    filler += q0_items(2) + q0_items(3)
                if pp + 1 < NPAIR:
                    filler += make_kq_filler(pp + 1)
                if pp == NPAIR - 1:
                    def unlock(sc):
                        items = []
                        for si in range(4):
                            items += wo_items(sc, si)
                        return items
                else:
                    unlock = no_unlock
                attention_pair(pp, filler, unlock)

    nc.compile()
    return nc


@functools.lru_cache(maxsize=1)
def _built():
    return _build()


def _prep_in_maps(x, Wq, bq, Wk, bk, Wv, bv, Wo, bo):
    f = np.float32
    x = np.asarray(x, f)
    Wq, bq = np.asarray(Wq, f), np.asarray(bq, f)
    Wk, bk = np.asarray(Wk, f), np.asarray(bk, f)
    Wv, bv = np.asarray(Wv, f), np.asarray(bv, f)
    Wo, bo = np.asarray(Wo, f), np.asarray(bo, f)
    h = np.float16
    ones = np.ones((1, 512), h)

    in_maps = []
    for core in range(NCORES):
        b, g = core // 2, core % 2
        h0, h1 = g * HLOC, (g + 1) * HLOC
        m = {
            "xT": np.ascontiguousarray(x[b].T.astype(h)),                         # [D, S]
            "wq": np.ascontiguousarray(Wq[h0:h1].transpose(1, 0, 2).reshape(D, -1).astype(h)),
            "wk": np.ascontiguousarray(Wk[h0:h1].transpose(1, 0, 2).reshape(D, -1).astype(h)),
            "wv": np.ascontiguousarray(Wv[h0:h1].transpose(1, 0, 2).reshape(D, -1).astype(h)),
            "wo": np.ascontiguousarray(Wo[h0:h1].reshape(HLOC * HD, D).astype(h)),
            "bq": np.ascontiguousarray(bq[h0:h1].reshape(NPAIR, 128).T),          # [128, 4]
            "bk": np.ascontiguousarray(bk[h0:h1].reshape(NPAIR, 128).T),
            "bv": bv[h0:h1].reshape(1, HLOC * HD).astype(h),
            "ones": ones,
        }
        in_maps.append(m)
    return in_maps


def kernel(x, Wq, bq, Wk, bk, Wv, bv, Wo, bo):
    from concourse.bass_utils import run_bass_kernel_spmd

    nc = _built()
    in_maps = _prep_in_maps(x, Wq, bq, Wk, bk, Wv, bv, Wo, bo)
    res = run_bass_kernel_spmd(nc, in_maps, list(range(NCORES)), trace=TRACE)
    LAST["exec_time_ns"] = res.exec_time_ns
    LAST["profile_json"] = res.profile_json

    bo32 = np.asarray(bo, np.float32)
    out = np.empty((B, S, D), np.float32)
    for b in range(B):
        out[b] = res.results[2 * b]["out"] + res.results[2 * b + 1]["out"] + bo32
    return out


# revision 38
# speedup vs baseline: 1.0039x; 1.0039x over previous
"""Multi-head attention (B=4, S=2048, D=1024, 16 heads x 64) on 8 trn2 cores.

Sharding: core = 2*b + g  (b: batch 0..3 data-parallel, g: head-group 0..1
tensor-parallel over 8 heads each).  Each core computes a partial
out[b] = softmax(q k^T / 8) v @ Wo[heads_g] (+ biases); the host sums the
two partials per batch.

Pipeline structure (491 us baseline -> 413 us):
  - x arrives in 8 di-chunk tiles; k/q projection for pair 0 runs
    di-outer (k all 4 sc + q sc0/1) paced by the chunk DMAs
  - v projection emitted just-in-time inside attention(0) sc0's t-loop;
    q sc2/3 + kq of pair p+1 emitted as filler items inside the
    attention t-loops, so the PE has independent work while it would
    otherwise stall on exp (ScalarE) results, and HAM stays at 8/8
  - ScalarE is the critical path: 256 exp activations over [128,1024]
    at ~1.34 us each ~= 343 us; the PE (~360 us of slots including
    drain overlap) hides underneath it
  - PSUM: scps 2x[128,1024] (4 banks) + afps 2x[65,512] (2) +
    mps 2x[128,512] (2) = 8 banks
"""

import sys
import functools

sys.path.insert(0, "/opt/trn_rl_repo")

import numpy as np

B, S, D = 4, 2048, 1024
NHEAD, HD = 16, 64
HLOC = 8          # heads per core
NPAIR = 4         # head pairs per core
NCORES = 8
NT = S // 128     # 16 t-blocks

TRACE = False     # test harness can flip this for profiling
LAST = {}         # exec_time_ns etc. from the most recent run


def _build():
    import concourse.tile as tile
    from concourse import bacc, mybir

    f32 = mybir.dt.float32
    f16 = mybir.dt.float16
    EXP = mybir.ActivationFunctionType.Exp

    nc = bacc.Bacc(None)

    xT_d = nc.dram_tensor("xT", [D, S], f16, kind="ExternalInput")
    wq_d = nc.dram_tensor("wq", [D, HLOC * HD], f16, kind="ExternalInput")
    wk_d = nc.dram_tensor("wk", [D, HLOC * HD], f16, kind="ExternalInput")
    wv_d = nc.dram_tensor("wv", [D, HLOC * HD], f16, kind="ExternalInput")
    wo_d = nc.dram_tensor("wo", [HLOC * HD, D], f16, kind="ExternalInput")
    bq_d = nc.dram_tensor("bq", [128, NPAIR], f32, kind="ExternalInput")
    bk_d = nc.dram_tensor("bk", [128, NPAIR], f32, kind="ExternalInput")
    bv_d = nc.dram_tensor("bv", [1, HLOC * HD], f16, kind="ExternalInput")
    ones_d = nc.dram_tensor("ones", [1, 512], f16, kind="ExternalInput")
    out_d = nc.dram_tensor("out", [S, D], f32, kind="ExternalOutput")

    wkr = wk_d.rearrange("(a p) (b c) -> a b p c", p=128, c=128)
    wqr = wq_d.rearrange("(a p) (b c) -> a b p c", p=128, c=128)
    wvr = wv_d.rearrange("(a p) c -> a p c", p=128)
    xTr = xT_d.rearrange("(n p) s -> n p s", p=128)
    wor = wo_d.rearrange("(p q) d -> p q d", q=128)

    with tile.TileContext(nc) as tc:
        with (
            tc.tile_pool(name="const", bufs=1) as const,
            tc.tile_pool(name="qkvres", bufs=1) as qkvres,
            tc.tile_pool(name="wp", bufs=20) as wp,
            tc.tile_pool(name="afp", bufs=1) as afp,
            tc.tile_pool(name="wop", bufs=1) as wop,
            tc.tile_pool(name="expp", bufs=4) as expp,
            tc.tile_pool(name="small", bufs=4) as small,
            tc.tile_pool(name="ostage", bufs=2) as ostage,
            tc.tile_pool(name="mps", bufs=2, space="PSUM") as mps,
            tc.tile_pool(name="scps", bufs=2, space="PSUM") as scps,
            tc.tile_pool(name="afps", bufs=2, space="PSUM") as afps,
        ):
            # ---- staged DMAs: pair-0 weights + x chunks first -------------
            wkq0 = []
            for di in range(8):
                wk_i = wp.tile([128, 128], f16, tag="wk0", name="wk0")
                nc.sync.dma_start(wk_i[:], wkr[di, 0])
                wq_i = wp.tile([128, 128], f16, tag="wq0", name="wq0")
                nc.sync.dma_start(wq_i[:], wqr[di, 0])
                wkq0.append((wk_i, wq_i))

            xt = []
            for di in range(8):
                xt_i = qkvres.tile([128, S], f16, name=f"xt{di}")
                nc.sync.dma_start(xt_i[:], xTr[di])
                xt.append(xt_i)

            ones = const.tile([1, 512], f16)
            nc.sync.dma_start(ones[:], ones_d[:])
            bqs = const.tile([128, NPAIR], f32)
            nc.sync.dma_start(bqs[:], bq_d[:])
            bks = const.tile([128, NPAIR], f32)
            nc.sync.dma_start(bks[:], bk_d[:])
            bvs = const.tile([1, HLOC * HD], f16)
            nc.sync.dma_start(bvs[:], bv_d[:])
            vcol = const.tile([128, HLOC], f32)
            nc.vector.memset(vcol[:], 1.0)

            wv_t = []
            for di in range(8):
                w_i = wp.tile([128, 512], f16, tag="w", name="wv_i")
                nc.sync.dma_start(w_i[:], wvr[di])
                wv_t.append(w_i)

            qT = [qkvres.tile([128, S], f16, name=f"qT{p}") for p in range(NPAIR)]
            kT = [qkvres.tile([128, S], f16, name=f"kT{p}") for p in range(NPAIR)]
            v_sb = [
                qkvres.tile([128, HLOC, HD + 1], f16, name=f"v{t}")
                for t in range(NT)
            ]
            af_sb = [afp.tile([128, S], f16, name=f"af{p}") for p in range(NPAIR)]
            wo_sb = wop.tile([128, NPAIR, D], f16)

            # ---- kq(0): di-outer, paced by the x chunk DMAs ---------------
            # k (all 4 sc, two scps tiles) + q sc0/1 (two mps tiles) in one
            # x sweep; q sc2/3 deferred to fillers inside attention(0).
            kg = [
                scps.tile([128, 1024], f32, tag="sc", name=f"kg{g}")
                for g in range(2)
            ]
            qg0 = [
                mps.tile([128, 512], f32, tag="ps", name=f"qg{sc}")
                for sc in range(2)
            ]
            for di in range(8):
                for g in range(2):
                    for h in range(2):
                        sc = 2 * g + h
                        nc.tensor.matmul(
                            kg[g][:, h * 512 : (h + 1) * 512],
                            wkq0[di][0][:],
                            xt[di][:, sc * 512 : (sc + 1) * 512],
                            start=(di == 0),
                            stop=(di == 7),
                        )
                for sc in range(2):
                    nc.tensor.matmul(
                        qg0[sc][:],
                        wkq0[di][1][:],
                        xt[di][:, sc * 512 : (sc + 1) * 512],
                        start=(di == 0),
                        stop=(di == 7),
                    )
            for g in range(2):
                for h in range(2):
                    sc = 2 * g + h
                    nc.vector.tensor_add(
                        kT[0][:, sc * 512 : (sc + 1) * 512],
                        kg[g][:, h * 512 : (h + 1) * 512],
                        bks[:, 0:1].broadcast_to([128, 512]),
                    )
            for sc in range(2):
                nc.vector.tensor_add(
                    qT[0][:, sc * 512 : (sc + 1) * 512],
                    qg0[sc][:],
                    bqs[:, 0:1].broadcast_to([128, 512]),
                )

            # ---- filler items ---------------------------------------------
            def v_block(t):
                """v projection for t-block t -> v_sb[t] (+ ones column)."""
                vp = mps.tile([128, 512], f32, tag="ps", name="vp")
                for di in range(8):
                    nc.tensor.matmul(
                        vp[:],
                        xt[di][:, t * 128 : (t + 1) * 128],
                        wv_t[di][:],
                        start=(di == 0),
                        stop=False,
                    )
                nc.tensor.matmul(
                    vp[:], ones[0:1, 0:128], bvs[:], start=False, stop=True
                )
                nc.vector.tensor_copy(
                    v_sb[t][:, :, 0:64],
                    vp[:].rearrange("p (n h) -> p n h", h=64),
                )
                nc.vector.tensor_copy(v_sb[t][:, :, 64:65], vcol[:, :, None])

            def proj_half(st, pp, wsel, sc, half, wblk):
                """Half of a 512-col k/q projection chunk for pair pp; the
                PSUM accumulator is shared between the two halves via st so
                each filler pop stays under ~1us of PE time and never
                delays the next scores by more than that."""
                dest, bias = ((kT, bks), (qT, bqs))[wsel]
                if half == 0:
                    st["p"] = mps.tile([128, 512], f32, tag="ps", name="pj")
                p = st["p"]
                for di in range(4 * half, 4 * half + 4):
                    nc.tensor.matmul(
                        p[:],
                        wblk[di][wsel][:],
                        xt[di][:, sc * 512 : (sc + 1) * 512],
                        start=(di == 0),
                        stop=(di == 7),
                    )
                if half == 1:
                    nc.vector.tensor_add(
                        dest[pp][:, sc * 512 : (sc + 1) * 512],
                        p[:],
                        bias[:, pp : pp + 1].broadcast_to([128, 512]),
                    )

            def proj_halves(pp, wsel, sc, wblk):
                st = {}
                return [
                    functools.partial(proj_half, st, pp, wsel, sc, h, wblk)
                    for h in range(2)
                ]

            def q0_items(sc):
                return proj_halves(0, 1, sc, wkq0)

            def make_kq_filler(pp):
                """DMA + projection half-items for pair pp (k then q)."""
                wblk = []
                for di in range(8):
                    wk_i = wp.tile([128, 128], f16, tag="wkn", name="wkN")
                    nc.sync.dma_start(wk_i[:], wkr[di, pp])
                    wq_i = wp.tile([128, 128], f16, tag="wqn", name="wqN")
                    nc.sync.dma_start(wq_i[:], wqr[di, pp])
                    wblk.append((wk_i, wq_i))
                items = []
                for wsel in range(2):
                    for sc in range(4):
                        items += proj_halves(pp, wsel, sc, wblk)
                return items

            def wo_half(st, sc, si, dch):
                """Half (one 512-wide output chunk) of a Wo output block."""
                s0 = sc * 512 + si * 128
                if dch == 0:
                    st["ot"] = ostage.tile([128, D], f32, tag="ost", name="ot")
                ot = st["ot"]
                op = mps.tile([128, 512], f32, tag="ps", name="op")
                for pp in range(NPAIR):
                    nc.tensor.matmul(
                        op[:],
                        af_sb[pp][:, s0 : s0 + 128],
                        wo_sb[:, pp, dch * 512 : (dch + 1) * 512],
                        start=(pp == 0),
                        stop=(pp == NPAIR - 1),
                    )
                nc.vector.tensor_copy(
                    ot[:, dch * 512 : (dch + 1) * 512], op[:]
                )
                if dch == 1:
                    nc.sync.dma_start(out_d[s0 : s0 + 128, :], ot[:])

            def wo_items(sc, si):
                st = {}
                return [
                    functools.partial(wo_half, st, sc, si, d) for d in range(2)
                ]

            # ---- attention with interleaved fillers -----------------------
            def attention_pair(pp, filler, unlock):
                fi = [0]

                def pop_filler():
                    if fi[0] < len(filler):
                        filler[fi[0]]()
                        fi[0] += 1

                slots = 3 * NT
                # pair 3 starts with an empty list that norm(sc) extends
                # with Wo half-items; poll those at a fixed cadence
                stride = max(1, slots // len(filler)) if filler else 2
                for sc in range(4):
                    ss = sc * 512
                    af0 = afps.tile([65, 512], f32, tag="afps", name="af0")
                    af1 = afps.tile([65, 512], f32, tag="afps", name="af1")
                    for t in range(NT):
                        scp = scps.tile([128, 1024], f32, tag="sc", name="scp")
                        nc.tensor.matmul(
                            scp[:, 0:512],
                            kT[pp][0:64, t * 128 : (t + 1) * 128],
                            qT[pp][0:64, ss : ss + 512],
                            start=True,
                            stop=True,
                            tile_position=(0, 0),
                        )
                        nc.tensor.matmul(
                            scp[:, 512:1024],
                            kT[pp][64:128, t * 128 : (t + 1) * 128],
                            qT[pp][64:128, ss : ss + 512],
                            start=True,
                            stop=True,
                            tile_position=(64, 0),
                        )
                        ex = expp.tile([128, 1024], f16, tag="ex", name="ex")
                        nc.scalar.activation(ex[:], scp[:], EXP, scale=0.125)
                        if pp == 0 and sc == 0:
                            v_block(t)  # just-in-time v for attention(0)
                        nc.tensor.matmul(
                            af0[:],
                            v_sb[t][:, 2 * pp, :],
                            ex[:, 0:512],
                            start=(t == 0),
                            stop=(t == NT - 1),
                        )
                        nc.tensor.matmul(
                            af1[:],
                            v_sb[t][:, 2 * pp + 1, :],
                            ex[:, 512:1024],
                            start=(t == 0),
                            stop=(t == NT - 1),
                        )
                        # interleave filler items across sc 1..3
                        if sc > 0:
                            if ((sc - 1) * NT + t) % stride == stride - 1:
                                pop_filler()
                    for rr, afx in ((0, af0), (1, af1)):
                        zrow = small.tile([1, 512], f32, tag="zrow", name="zrow")
                        nc.vector.tensor_copy(zrow[:], afx[64:65, :])
                        zb = small.tile([64, 512], f32, tag="zb", name="zb")
                        nc.gpsimd.partition_broadcast(zb[:], zrow[:])
                        rec64 = small.tile([64, 512], f32, tag="rec", name="rec64")
                        nc.vector.reciprocal_approx_fast(rec64[:], zb[:])
                        nc.vector.tensor_mul(
                            af_sb[pp][64 * rr : 64 * rr + 64, ss : ss + 512],
                            afx[0:64, :],
                            rec64[:],
                        )
                    filler.extend(unlock(sc))
                while fi[0] < len(filler):
                    filler[fi[0]]()
                    fi[0] += 1

            for pp in range(NPAIR):
                nc.sync.dma_start(wo_sb[:, pp, :], wor[pp])

            def no_unlock(sc):
                return []

            for pp in range(NPAIR):
                filler = []
                if pp == 0:
                    filler += q0_items(2) + q0_items(3)
                if pp + 1 < NPAIR:
                    filler += make_kq_filler(pp + 1)
                if pp == NPAIR - 1:
                    def unlock(sc):
                        items = []
                        for si in range(4):
                            items += wo_items(sc, si)
                        return items
                else:
                    unlock = no_unlock
                attention_pair(pp, filler, unlock)

    nc.compile()
    return nc


@functools.lru_cache(maxsize=1)
def _built():
    return _build()


def _prep_in_maps(x, Wq, bq, Wk, bk, Wv, bv, Wo, bo):
    f = np.float32
    x = np.asarray(x, f)
    Wq, bq = np.asarray(Wq, f), np.asarray(bq, f)
    Wk, bk = np.asarray(Wk, f), np.asarray(bk, f)
    Wv, bv = np.asarray(Wv, f), np.asarray(bv, f)
    Wo, bo = np.asarray(Wo, f), np.asarray(bo, f)
    h = np.float16
    ones = np.ones((1, 512), h)

    in_maps = []
    for core in range(NCORES):
        b, g = core // 2, core % 2
        h0, h1 = g * HLOC, (g + 1) * HLOC
        m = {
            "xT": np.ascontiguousarray(x[b].T.astype(h)),                         # [D, S]
            "wq": np.ascontiguousarray(Wq[h0:h1].transpose(1, 0, 2).reshape(D, -1).astype(h)),
            "wk": np.ascontiguousarray(Wk[h0:h1].transpose(1, 0, 2).reshape(D, -1).astype(h)),
            "wv": np.ascontiguousarray(Wv[h0:h1].transpose(1, 0, 2).reshape(D, -1).astype(h)),
            "wo": np.ascontiguousarray(Wo[h0:h1].reshape(HLOC * HD, D).astype(h)),
            "bq": np.ascontiguousarray(bq[h0:h1].reshape(NPAIR, 128).T),          # [128, 4]
            "bk": np.ascontiguousarray(bk[h0:h1].reshape(NPAIR, 128).T),
            "bv": bv[h0:h1].reshape(1, HLOC * HD).astype(h),
            "ones": ones,
        }
        in_maps.append(m)
    return in_maps


def kernel(x, Wq, bq, Wk, bk, Wv, bv, Wo, bo):
    from concourse.bass_utils import run_bass_kernel_spmd

    nc = _built()
    in_maps = _prep_in_maps(x, Wq, bq, Wk, bk, Wv, bv, Wo, bo)
    res = run_bass_kernel_spmd(nc, in_maps, list(range(NCORES)), trace=TRACE)
    LAST["exec_time_ns"] = res.exec_time_ns
    LAST["profile_json"] = res.profile_json

    bo32 = np.asarray(bo, np.float32)
    out = np.empty((B, S, D), np.float32)
    for b in range(B):
        out[b] = res.results[2 * b]["out"] + res.results[2 * b + 1]["out"] + bo32
    return out


# revision 39
# speedup vs baseline: 1.0667x; 1.0625x over previous
"""Multi-head attention (B=4, S=2048, D=1024, 16 heads x 64) on 8 trn2 cores.

Sharding: core = 2*b + g  (b: batch 0..3 data-parallel, g: head-group 0..1
tensor-parallel over 8 heads each).  Each core computes a partial
out[b] = softmax(q k^T / 8) v @ Wo[heads_g] (+ biases); the host sums the
two partials per batch.

Pipeline structure (491 us baseline -> 413 us):
  - x arrives in 8 di-chunk tiles; k/q projection for pair 0 runs
    di-outer (k all 4 sc + q sc0/1) paced by the chunk DMAs
  - v projection emitted just-in-time inside attention(0) sc0's t-loop;
    q sc2/3 + kq of pair p+1 emitted as filler items inside the
    attention t-loops, so the PE has independent work while it would
    otherwise stall on exp (ScalarE) results, and HAM stays at 8/8
  - ScalarE is the critical path: 256 exp activations over [128,1024]
    at ~1.34 us each ~= 343 us; the PE (~360 us of slots including
    drain overlap) hides underneath it
  - PSUM: scps 2x[128,1024] (4 banks) + afps 2x[65,512] (2) +
    mps 2x[128,512] (2) = 8 banks
"""

import sys
import functools

sys.path.insert(0, "/opt/trn_rl_repo")

import numpy as np

B, S, D = 4, 2048, 1024
NHEAD, HD = 16, 64
HLOC = 8          # heads per core
NPAIR = 4         # head pairs per core
NCORES = 8
NT = S // 128     # 16 t-blocks

TRACE = False     # test harness can flip this for profiling
LAST = {}         # exec_time_ns etc. from the most recent run


def _build():
    import concourse.tile as tile
    from concourse import bacc, mybir

    f32 = mybir.dt.float32
    f16 = mybir.dt.float16
    EXP = mybir.ActivationFunctionType.Exp

    nc = bacc.Bacc(None)

    xT_d = nc.dram_tensor("xT", [D, S], f16, kind="ExternalInput")
    wq_d = nc.dram_tensor("wq", [D, HLOC * HD], f16, kind="ExternalInput")
    wk_d = nc.dram_tensor("wk", [D, HLOC * HD], f16, kind="ExternalInput")
    wv_d = nc.dram_tensor("wv", [D, HLOC * HD], f16, kind="ExternalInput")
    wo_d = nc.dram_tensor("wo", [HLOC * HD, D], f16, kind="ExternalInput")
    bq_d = nc.dram_tensor("bq", [128, NPAIR], f32, kind="ExternalInput")
    bk_d = nc.dram_tensor("bk", [128, NPAIR], f32, kind="ExternalInput")
    bv_d = nc.dram_tensor("bv", [1, HLOC * HD], f16, kind="ExternalInput")
    ones_d = nc.dram_tensor("ones", [1, 512], f16, kind="ExternalInput")
    out_d = nc.dram_tensor("out", [S, D], f32, kind="ExternalOutput")

    wkr = wk_d.rearrange("(a p) (b c) -> a b p c", p=128, c=128)
    wqr = wq_d.rearrange("(a p) (b c) -> a b p c", p=128, c=128)
    wvr = wv_d.rearrange("(a p) c -> a p c", p=128)
    xTr = xT_d.rearrange("(n p) s -> n p s", p=128)
    wor = wo_d.rearrange("(p q) d -> p q d", q=128)

    with tile.TileContext(nc) as tc:
        with (
            tc.tile_pool(name="const", bufs=1) as const,
            tc.tile_pool(name="qkvres", bufs=1) as qkvres,
            tc.tile_pool(name="wp", bufs=20) as wp,
            tc.tile_pool(name="afp", bufs=1) as afp,
            tc.tile_pool(name="wop", bufs=1) as wop,
            tc.tile_pool(name="expp", bufs=4) as expp,
            tc.tile_pool(name="small", bufs=4) as small,
            tc.tile_pool(name="ostage", bufs=2) as ostage,
            tc.tile_pool(name="mps", bufs=2, space="PSUM") as mps,
            tc.tile_pool(name="scps", bufs=2, space="PSUM") as scps,
            tc.tile_pool(name="afps", bufs=2, space="PSUM") as afps,
        ):
            # ---- staged DMAs: pair-0 weights + x chunks first -------------
            wkq0 = []
            for di in range(8):
                wk_i = wp.tile([128, 128], f16, tag="wk0", name="wk0")
                nc.sync.dma_start(wk_i[:], wkr[di, 0])
                wq_i = wp.tile([128, 128], f16, tag="wq0", name="wq0")
                nc.sync.dma_start(wq_i[:], wqr[di, 0])
                wkq0.append((wk_i, wq_i))

            xt = []
            for di in range(8):
                xt_i = qkvres.tile([128, S], f16, name=f"xt{di}")
                nc.sync.dma_start(xt_i[:], xTr[di])
                xt.append(xt_i)

            ones = const.tile([1, 512], f16)
            nc.sync.dma_start(ones[:], ones_d[:])
            bqs = const.tile([128, NPAIR], f32)
            nc.sync.dma_start(bqs[:], bq_d[:])
            bks = const.tile([128, NPAIR], f32)
            nc.sync.dma_start(bks[:], bk_d[:])
            bvs = const.tile([1, HLOC * HD], f16)
            nc.sync.dma_start(bvs[:], bv_d[:])
            vcol = const.tile([128, HLOC], f32)
            nc.vector.memset(vcol[:], 1.0)

            wv_t = []
            for di in range(8):
                w_i = wp.tile([128, 512], f16, tag="w", name="wv_i")
                nc.sync.dma_start(w_i[:], wvr[di])
                wv_t.append(w_i)

            qT = [qkvres.tile([128, S], f16, name=f"qT{p}") for p in range(NPAIR)]
            kT = [qkvres.tile([128, S], f16, name=f"kT{p}") for p in range(NPAIR)]
            v_sb = [
                qkvres.tile([128, HLOC, HD + 1], f16, name=f"v{t}")
                for t in range(NT)
            ]
            af_sb = [afp.tile([128, S], f16, name=f"af{p}") for p in range(NPAIR)]
            wo_sb = wop.tile([128, NPAIR, D], f16)

            # ---- kq(0): di-outer, paced by the x chunk DMAs ---------------
            # k (all 4 sc, two scps tiles) + q sc0/1 (two mps tiles) in one
            # x sweep; q sc2/3 deferred to fillers inside attention(0).
            kg = [
                scps.tile([128, 1024], f32, tag="sc", name=f"kg{g}")
                for g in range(2)
            ]
            qg0 = [
                mps.tile([128, 512], f32, tag="ps", name=f"qg{sc}")
                for sc in range(2)
            ]
            for di in range(8):
                for g in range(2):
                    for h in range(2):
                        sc = 2 * g + h
                        nc.tensor.matmul(
                            kg[g][:, h * 512 : (h + 1) * 512],
                            wkq0[di][0][:],
                            xt[di][:, sc * 512 : (sc + 1) * 512],
                            start=(di == 0),
                            stop=(di == 7),
                        )
                for sc in range(2):
                    nc.tensor.matmul(
                        qg0[sc][:],
                        wkq0[di][1][:],
                        xt[di][:, sc * 512 : (sc + 1) * 512],
                        start=(di == 0),
                        stop=(di == 7),
                    )
            for g in range(2):
                for h in range(2):
                    sc = 2 * g + h
                    nc.vector.tensor_add(
                        kT[0][:, sc * 512 : (sc + 1) * 512],
                        kg[g][:, h * 512 : (h + 1) * 512],
                        bks[:, 0:1].broadcast_to([128, 512]),
                    )
            for sc in range(2):
                nc.vector.tensor_add(
                    qT[0][:, sc * 512 : (sc + 1) * 512],
                    qg0[sc][:],
                    bqs[:, 0:1].broadcast_to([128, 512]),
                )

            # ---- filler items ---------------------------------------------
            def v_block(t):
                """v projection for t-block t -> v_sb[t] (+ ones column)."""
                vp = mps.tile([128, 512], f32, tag="ps", name="vp")
                for di in range(8):
                    nc.tensor.matmul(
                        vp[:],
                        xt[di][:, t * 128 : (t + 1) * 128],
                        wv_t[di][:],
                        start=(di == 0),
                        stop=False,
                    )
                nc.tensor.matmul(
                    vp[:], ones[0:1, 0:128], bvs[:], start=False, stop=True
                )
                nc.vector.tensor_copy(
                    v_sb[t][:, :, 0:64],
                    vp[:].rearrange("p (n h) -> p n h", h=64),
                )
                nc.vector.tensor_copy(v_sb[t][:, :, 64:65], vcol[:, :, None])

            def proj_half(st, pp, wsel, sc, half, wblk):
                """Half of a 512-col k/q projection chunk for pair pp; the
                PSUM accumulator is shared between the two halves via st so
                each filler pop stays under ~1us of PE time and never
                delays the next scores by more than that."""
                dest, bias = ((kT, bks), (qT, bqs))[wsel]
                if half == 0:
                    st["p"] = mps.tile([128, 512], f32, tag="ps", name="pj")
                p = st["p"]
                for di in range(4 * half, 4 * half + 4):
                    nc.tensor.matmul(
                        p[:],
                        wblk[di][wsel][:],
                        xt[di][:, sc * 512 : (sc + 1) * 512],
                        start=(di == 0),
                        stop=(di == 7),
                    )
                if half == 1:
                    nc.vector.tensor_add(
                        dest[pp][:, sc * 512 : (sc + 1) * 512],
                        p[:],
                        bias[:, pp : pp + 1].broadcast_to([128, 512]),
                    )

            def proj_halves(pp, wsel, sc, wblk):
                st = {}
                return [
                    functools.partial(proj_half, st, pp, wsel, sc, h, wblk)
                    for h in range(2)
                ]

            def q0_items(sc):
                return proj_halves(0, 1, sc, wkq0)

            def make_kq_filler(pp):
                """DMA + projection half-items for pair pp (k then q)."""
                wblk = []
                for di in range(8):
                    wk_i = wp.tile([128, 128], f16, tag="wkn", name="wkN")
                    nc.sync.dma_start(wk_i[:], wkr[di, pp])
                    wq_i = wp.tile([128, 128], f16, tag="wqn", name="wqN")
                    nc.sync.dma_start(wq_i[:], wqr[di, pp])
                    wblk.append((wk_i, wq_i))
                items = []
                for wsel in range(2):
                    for sc in range(4):
                        items += proj_halves(pp, wsel, sc, wblk)
                return items

            def wo_half(st, sc, si, dch):
                """Half (one 512-wide output chunk) of a Wo output block."""
                s0 = sc * 512 + si * 128
                if dch == 0:
                    st["ot"] = ostage.tile([128, D], f32, tag="ost", name="ot")
                ot = st["ot"]
                op = mps.tile([128, 512], f32, tag="ps", name="op")
                for pp in range(NPAIR):
                    nc.tensor.matmul(
                        op[:],
                        af_sb[pp][:, s0 : s0 + 128],
                        wo_sb[:, pp, dch * 512 : (dch + 1) * 512],
                        start=(pp == 0),
                        stop=(pp == NPAIR - 1),
                    )
                nc.vector.tensor_copy(
                    ot[:, dch * 512 : (dch + 1) * 512], op[:]
                )
                if dch == 1:
                    nc.sync.dma_start(out_d[s0 : s0 + 128, :], ot[:])

            def wo_items(sc, si):
                st = {}
                return [
                    functools.partial(wo_half, st, sc, si, d) for d in range(2)
                ]

            # ---- attention with interleaved fillers -----------------------
            def attention_pair(pp, filler, unlock):
                fi = [0]

                def pop_filler():
                    if fi[0] < len(filler):
                        filler[fi[0]]()
                        fi[0] += 1

                slots = 3 * NT
                # pair 3 starts with an empty list that norm(sc) extends
                # with Wo half-items; poll those at a fixed cadence
                stride = max(1, slots // len(filler)) if filler else 2
                for sc in range(4):
                    ss = sc * 512
                    af0 = afps.tile([65, 512], f32, tag="afps", name="af0")
                    af1 = afps.tile([65, 512], f32, tag="afps", name="af1")
                    for t in range(NT):
                        scp = scps.tile([128, 1024], f32, tag="sc", name="scp")
                        nc.tensor.matmul(
                            scp[:, 0:512],
                            kT[pp][0:64, t * 128 : (t + 1) * 128],
                            qT[pp][0:64, ss : ss + 512],
                            start=True,
                            stop=True,
                            tile_position=(0, 0),
                        )
                        nc.tensor.matmul(
                            scp[:, 512:1024],
                            kT[pp][64:128, t * 128 : (t + 1) * 128],
                            qT[pp][64:128, ss : ss + 512],
                            start=True,
                            stop=True,
                            tile_position=(64, 0),
                        )
                        ex = expp.tile([128, 1024], f16, tag="ex", name="ex")
                        nc.scalar.activation(ex[:], scp[:], EXP, scale=0.125)
                        if pp == 0 and sc == 0:
                            v_block(t)  # just-in-time v for attention(0)
                        nc.tensor.matmul(
                            af0[:],
                            v_sb[t][:, 2 * pp, :],
                            ex[:, 0:512],
                            start=(t == 0),
                            stop=(t == NT - 1),
                        )
                        nc.tensor.matmul(
                            af1[:],
                            v_sb[t][:, 2 * pp + 1, :],
                            ex[:, 512:1024],
                            start=(t == 0),
                            stop=(t == NT - 1),
                        )
                        # interleave filler items across sc 1..3
                        if sc > 0:
                            if ((sc - 1) * NT + t) % stride == stride - 1:
                                pop_filler()
                    for rr, afx in ((0, af0), (1, af1)):
                        # evacuate the PSUM accumulator with two quick
                        # copies so the af ring frees ~2.5us sooner at sc
                        # boundaries; normalize from the SBUF staging copy
                        zrow = small.tile([1, 512], f32, tag="zrow", name="zrow")
                        nc.vector.tensor_copy(zrow[:], afx[64:65, :])
                        stg = small.tile([64, 512], f32, tag="stg", name="stg")
                        nc.vector.tensor_copy(stg[:], afx[0:64, :])
                        zb = small.tile([64, 512], f32, tag="zb", name="zb")
                        nc.gpsimd.partition_broadcast(zb[:], zrow[:])
                        rec64 = small.tile([64, 512], f32, tag="rec", name="rec64")
                        nc.vector.reciprocal_approx_fast(rec64[:], zb[:])
                        nc.vector.tensor_mul(
                            af_sb[pp][64 * rr : 64 * rr + 64, ss : ss + 512],
                            stg[:],
                            rec64[:],
                        )
                    filler.extend(unlock(sc))
                while fi[0] < len(filler):
                    filler[fi[0]]()
                    fi[0] += 1

            for pp in range(NPAIR):
                nc.sync.dma_start(wo_sb[:, pp, :], wor[pp])

            def no_unlock(sc):
                return []

            for pp in range(NPAIR):
                filler = []
                if pp == 0:
                    filler += q0_items(2) + q0_items(3)
                if pp + 1 < NPAIR:
                    filler += make_kq_filler(pp + 1)
                if pp == NPAIR - 1:
                    def unlock(sc):
                        items = []
                        for si in range(4):
                            items += wo_items(sc, si)
                        return items
                else:
                    unlock = no_unlock
                attention_pair(pp, filler, unlock)

    nc.compile()
    return nc


@functools.lru_cache(maxsize=1)
def _built():
    return _build()


def _prep_in_maps(x, Wq, bq, Wk, bk, Wv, bv, Wo, bo):
    f = np.float32
    x = np.asarray(x, f)
    Wq, bq = np.asarray(Wq, f), np.asarray(bq, f)
    Wk, bk = np.asarray(Wk, f), np.asarray(bk, f)
    Wv, bv = np.asarray(Wv, f), np.asarray(bv, f)
    Wo, bo = np.asarray(Wo, f), np.asarray(bo, f)
    h = np.float16
    ones = np.ones((1, 512), h)

    in_maps = []
    for core in range(NCORES):
        b, g = core // 2, core % 2
        h0, h1 = g * HLOC, (g + 1) * HLOC
        m = {
            "xT": np.ascontiguousarray(x[b].T.astype(h)),                         # [D, S]
            "wq": np.ascontiguousarray(Wq[h0:h1].transpose(1, 0, 2).reshape(D, -1).astype(h)),
            "wk": np.ascontiguousarray(Wk[h0:h1].transpose(1, 0, 2).reshape(D, -1).astype(h)),
            "wv": np.ascontiguousarray(Wv[h0:h1].transpose(1, 0, 2).reshape(D, -1).astype(h)),
            "wo": np.ascontiguousarray(Wo[h0:h1].reshape(HLOC * HD, D).astype(h)),
            "bq": np.ascontiguousarray(bq[h0:h1].reshape(NPAIR, 128).T),          # [128, 4]
            "bk": np.ascontiguousarray(bk[h0:h1].reshape(NPAIR, 128).T),
            "bv": bv[h0:h1].reshape(1, HLOC * HD).astype(h),
            "ones": ones,
        }
        in_maps.append(m)
    return in_maps


def kernel(x, Wq, bq, Wk, bk, Wv, bv, Wo, bo):
    from concourse.bass_utils import run_bass_kernel_spmd

    nc = _built()
    in_maps = _prep_in_maps(x, Wq, bq, Wk, bk, Wv, bv, Wo, bo)
    res = run_bass_kernel_spmd(nc, in_maps, list(range(NCORES)), trace=TRACE)
    LAST["exec_time_ns"] = res.exec_time_ns
    LAST["profile_json"] = res.profile_json

    bo32 = np.asarray(bo, np.float32)
    out = np.empty((B, S, D), np.float32)
    for b in range(B):
        out[b] = res.results[2 * b]["out"] + res.results[2 * b + 1]["out"] + bo32
    return out
